# revision 4
# baseline (speedup 1.0000x reference)
"""ConsensusAttention Trainium2 kernel.

Reference computation (per (b, l) of levels (8, 1024, 6, 512)):
    k = levels * rsqrt(max(sum(levels^2), 1e-12))          # GLOBAL l2 scale
    sim[b,l,i,j] = (q_i . k_j) / sqrt(512)
    sim diag <- -0.0005 ; sim[dist(i,j) > 2] <- -FLT_MAX   # 32x32 grid, radius 2
    out = softmax(sim) @ v

Strategy: data-parallel over batch (8 cores, one batch each). Per (b, l) the
attention matrix is a banded matrix (13 grid-neighbor diagonals, offsets in
[-66, 66]); we compute 128-query tiles against 384-wide aligned key slabs.
Logits are tiny (|sim| < ~0.01) so softmax needs no max subtraction; masked
entries use an additive -1e38 mask so exp underflows to exactly 0. The
attention band is symmetric (S = X X^T, symmetric masks), so the transposed
A-blocks needed by the A@V matmul are exactly the A-blocks computed by the
neighboring query tiles - no on-chip transposition of A is needed.

Two NEFF launches: (1) per-core partial sum-of-squares of its shard (the only
cross-core quantity is this single scalar; the 8 partials are combined on the
host, which also does the shard/gather glue), (2) the attention kernel with
the combined scale passed as a (128, 1) broadcast input.
"""

import math
import os

import numpy as np

B, N, L, D = 8, 1024, 6, 512
GRID = 32            # 32x32 patch grid, row-major
RADIUS2 = 4          # radius 2.0 squared
SELF_LOGIT = -0.0005
L2_EPS = 1e-12
P = 128              # partitions
NT = N // P          # 8 query tiles per (b, l)
W = 3 * P            # 384-wide key slab (aligned tiles t-1, t, t+1)
NEG = -1e38

_cache: dict = {}


def _masks():
    """Additive mask (NT, P, W): 0 on valid off-diagonal neighbors, -1e38 on
    diagonal / non-neighbors / out-of-range."""
    i = np.arange(N)
    hi, wi = i // GRID, i % GRID
    m = np.full((NT, P, W), NEG, np.float32)
    for t in range(NT):
        iq = t * P + np.arange(P)
        jk = (t - 1) * P + np.arange(W)
        valid = (jk >= 0) & (jk < N)
        jj = np.clip(jk, 0, N - 1)
        dh = hi[iq][:, None] - hi[jj][None, :]
        dw = wi[iq][:, None] - wi[jj][None, :]
        keep = (dh * dh + dw * dw <= RADIUS2) & valid[None, :] & (iq[:, None] != jj[None, :])
        m[t][keep] = 0.0
    return m


def _build_norm_nc():
    import concourse.mybir as mybir
    import concourse.tile as tile
    from concourse import bacc

    F = mybir.dt.float32
    nc = bacc.Bacc("TRN2", target_bir_lowering=False, debug=False,
                   enable_asserts=True, num_devices=8)
    x = nc.dram_tensor("x", [N, L * D], F, kind="ExternalInput").ap()
    ss = nc.dram_tensor("ss", [P, NT], F, kind="ExternalOutput").ap()
    xr = x.rearrange("(j p) m -> p j m", p=P)

    with tile.TileContext(nc) as tc:
        with tc.tile_pool(name="sb", bufs=3) as sb, \
             tc.tile_pool(name="scr", bufs=2) as scr:
            acc = sb.tile([P, NT], F)
            for j in range(NT):
                xt = sb.tile([P, L * D], F, tag="xt")
                nc.sync.dma_start(xt[:], xr[:, j, :])
                sq = scr.tile([P, L * D], F, tag="sq")
                nc.scalar.activation(sq[:], xt[:], mybir.ActivationFunctionType.Square,
                                     bias=0.0, scale=1.0, accum_out=acc[:, j:j + 1])
            nc.sync.dma_start(ss, acc[:])
    nc.compile()
    return nc


def _build_attn_nc():
    import concourse.mybir as mybir
    import concourse.tile as tile
    from concourse import bacc

    F = mybir.dt.float32
    FR = mybir.dt.float32r
    AF = mybir.ActivationFunctionType

    nc = bacc.Bacc("TRN2", target_bir_lowering=False, debug=False,
                   enable_asserts=True, num_devices=8)
    # x declared float32r: same bits, lets DMA-loaded tiles feed f32r matmuls
    x = nc.dram_tensor("x", [N, L, D], FR, kind="ExternalInput").ap()
    m2 = nc.dram_tensor("m2", [NT, P, W], F, kind="ExternalInput").ap()
    ident = nc.dram_tensor("ident", [P, P], FR, kind="ExternalInput").ap()
    di = nc.dram_tensor("di", [P, P], F, kind="ExternalInput").ap()
    c_in = nc.dram_tensor("c", [P, 1], F, kind="ExternalInput").ap()
    o = nc.dram_tensor("o", [N, L, D], F, kind="ExternalOutput").ap()

    xr = x.rearrange("(j p) l d -> p j l d", p=P)
    orr = o.rearrange("(j p) l d -> p j l d", p=P)

    with tile.TileContext(nc) as tc:
        with tc.tile_pool(name="const", bufs=1) as cst, \
             tc.tile_pool(name="data", bufs=2) as data, \
             tc.tile_pool(name="outp", bufs=4) as outp, \
             tc.tile_pool(name="ps_xt", bufs=2, space="PSUM") as ps_xt, \
             tc.tile_pool(name="ps_s", bufs=2, space="PSUM") as ps_s, \
             tc.tile_pool(name="ps_o", bufs=2, space="PSUM") as ps_o:

            id_sb = cst.tile([P, P], FR)
            nc.sync.dma_start(id_sb[:], ident)
            m2_sb = cst.tile([P, NT, W], F)
            nc.sync.dma_start(m2_sb[:], m2.rearrange("t p w -> p t w"))
            c_sb = cst.tile([P, 1], F)
            nc.sync.dma_start(c_sb[:], c_in)
            di_sb = cst.tile([P, P], F)
            nc.sync.dma_start(di_sb[:], di)
            di_r = cst.tile([P, P], FR)
            nc.vector.tensor_copy(di_r[:], di_sb[:])

            for l in range(L):
                x_sb = data.tile([P, NT, D], FR, tag="x")
                nc.sync.dma_start(x_sb[:], xr[:, :, l, :])

                # XT[p, kd, tok] = X[tok, kd*128+p]
                xt_sb = data.tile([P, D // P, N], FR, tag="xt")
                for j in range(NT):
                    tp = ps_xt.tile([P, D], FR, tag="tps")
                    for kd in range(D // P):
                        nc.tensor.transpose(tp[:, kd * P:(kd + 1) * P],
                                            x_sb[:, j, kd * P:(kd + 1) * P], id_sb[:])
                    nc.vector.tensor_copy(
                        xt_sb[:, :, j * P:(j + 1) * P],
                        tp[:].rearrange("p (kd q) -> p kd q", kd=D // P))

                a_sb = data.tile([P, NT, W], FR, tag="a")
                dsum = data.tile([P, NT], F, tag="dsum")
                for t in range(NT):
                    lo, hi = max(t - 1, 0), min(t + 2, NT)
                    c0, c1 = (lo - t + 1) * P, (hi - t + 1) * P
                    s_ps = ps_s.tile([P, W], F, tag="sps")
                    for kd in range(D // P):
                        nc.tensor.matmul(s_ps[:, c0:c1],
                                         xt_sb[:, kd, t * P:(t + 1) * P],
                                         xt_sb[:, kd, lo * P:hi * P],
                                         start=(kd == 0), stop=(kd == D // P - 1))
                    nc.vector.tensor_tensor(s_ps[:, c0:c1], s_ps[:, c0:c1],
                                            m2_sb[:, t, c0:c1], mybir.AluOpType.add)
                    nc.scalar.activation(a_sb[:, t, c0:c1], s_ps[:, c0:c1], AF.Exp,
                                         bias=0.0, scale=c_sb[:],
                                         accum_out=dsum[:, t:t + 1])

                denom = data.tile([P, NT], F, tag="den")
                nc.vector.tensor_scalar_add(denom[:], dsum[:], float(np.exp(SELF_LOGIT)))
                recip = data.tile([P, NT], F, tag="rec")
                nc.vector.reciprocal(recip[:], denom[:])

                for t in range(NT):
                    o_ps = ps_o.tile([P, D], F, tag="ops")
                    ks = [k for k in (t - 1, t, t + 1) if 0 <= k < NT]
                    for r, k in enumerate(ks):
                        # block(k, t) = A rows of query-tile k, key-tile t
                        blk = a_sb[:, k, (t - k + 1) * P:(t - k + 2) * P]
                        nc.tensor.matmul(o_ps[:], blk, x_sb[:, k, :],
                                         start=(r == 0), stop=False)
                    nc.tensor.matmul(o_ps[:], di_r[:], x_sb[:, t, :],
                                     start=False, stop=True)
                    out_sb = outp.tile([P, D], F, tag="o")
                    nc.scalar.activation(out_sb[:], o_ps[:], AF.Copy,
                                         bias=0.0, scale=recip[:, t:t + 1])
                    nc.sync.dma_start(orr[:, t, l, :], out_sb[:])
    nc.compile()
    return nc


def _get_progs():
    if "progs" not in _cache:
        _cache["progs"] = (_build_norm_nc(), _build_attn_nc())
    return _cache["progs"]


def _install_ntff_hook():
    """The agent image's antenv package lacks axon_hooks; recreate the NTFF
    profile hook (ctypes into libaxon_pjrt.so) and register it so
    run_bass_kernel_spmd(trace=True) can capture profiles. Only used by the
    local test harness (KERNEL_TRACE=1); never on the default path."""
    if _cache.get("hook_installed"):
        return
    import contextlib
    import ctypes
    import sys
    import types

    so_path = "/opt/axon/libaxon_pjrt.so"
    lib = ctypes.CDLL(so_path)
    lib.axon_start_nrt_profile.argtypes = [ctypes.POINTER(ctypes.c_int64), ctypes.c_size_t]
    lib.axon_start_nrt_profile.restype = ctypes.c_int64
    lib.axon_stop_nrt_profile.argtypes = [ctypes.c_char_p]
    lib.axon_stop_nrt_profile.restype = ctypes.c_int64

    @contextlib.contextmanager
    def _hook(output_dir, device_ids):
        import jax
        jax.devices()
        if device_ids:
            ids = (ctypes.c_int64 * len(device_ids))(*device_ids)
            rc = lib.axon_start_nrt_profile(ids, len(device_ids))
        else:
            rc = lib.axon_start_nrt_profile(None, 0)
        if rc != 0:
            raise RuntimeError(f"axon_start_nrt_profile rc={rc}")
        try:
            yield
        finally:
            n = lib.axon_stop_nrt_profile(str(output_dir).encode())
            print(f"ntff profile: {n} file(s) written to {output_dir}", file=sys.stderr)

    mod = types.ModuleType("antenv.axon_hooks")
    mod.get_axon_ntff_profile_hook = lambda: _hook
    mod.set_axon_ntff_profile_hook = lambda h: None
    import antenv
    antenv.axon_hooks = mod
    sys.modules["antenv.axon_hooks"] = mod
    _cache["hook_installed"] = True


last_exec_time_ns = {"norm": None, "attn": None}


def kernel(levels: np.ndarray) -> np.ndarray:
    from concourse.bass_utils import run_bass_kernel_spmd

    assert levels.shape == (B, N, L, D) and levels.dtype == np.float32
    norm_nc, attn_nc = _get_progs()
    trace = os.environ.get("KERNEL_TRACE", "0") == "1"
    if trace:
        try:
            _install_ntff_hook()
        except Exception as e:
            print(f"ntff hook unavailable ({e}); tracing disabled")
            trace = False
    cores = list(range(8))

    def _run(nc, maps):
        if trace:
            try:
                return run_bass_kernel_spmd(nc, maps, core_ids=cores, trace=True)
            except Exception as e:
                print(f"traced run failed ({e}); retrying untraced")
        return run_bass_kernel_spmd(nc, maps, core_ids=cores)

    shards = [np.ascontiguousarray(levels[b]) for b in range(B)]

    # Phase 1: per-core partial sum of squares of its shard.
    in1 = [{"x": s.reshape(N, L * D)} for s in shards]
    r1 = _run(norm_nc, in1)
    last_exec_time_ns["norm"] = r1.exec_time_ns
    ssq = float(np.sum([r.get("ss").astype(np.float64).sum() for r in r1.results]))
    g = 1.0 / math.sqrt(max(ssq, L2_EPS))
    c = np.float32(g / math.sqrt(D))

    # Phase 2: attention.
    consts = _consts()
    in2 = [{"x": s, "m2": consts["m2"], "ident": consts["ident"],
            "di": consts["di"], "c": np.full((P, 1), c, np.float32)} for s in shards]
    r2 = _run(attn_nc, in2)
    last_exec_time_ns["attn"] = r2.exec_time_ns

    return np.stack([r.get("o") for r in r2.results], axis=0)


def _consts():
    if "consts" not in _cache:
        _cache["consts"] = {
            "m2": _masks(),
            "ident": np.eye(P, dtype=np.float32),
            "di": np.float32(np.exp(SELF_LOGIT)) * np.eye(P, dtype=np.float32),
        }
    return _cache["consts"]
